# revision 5
# baseline (speedup 1.0000x reference)
"""Trainium2 Bass kernel for nn_EvolvedNet (gnn_message_passing).

Reference semantics: vals = zeros[32, B]; vals[:8] = x; then 32 sweeps
over 128 edges applied sequentially: vals[dst] += tanh(vals[src] * w);
output = tanh(vals[28:32]).

Strategy (per core, batch shard 65536 = [128 partitions x 512 free] f32):
  - Pure data parallel over 8 NeuronCores.
  - Host-side exact pruning of dead edge applications (source identically
    zero / result cannot reach an output).
  - Each node's state lives in SBUF, except the 8 highest in-degree nodes
    which live in PSUM banks and are accumulated by the Tensor engine
    (fp32 identity-matmul accumulate) - this offloads ~45% of the adds
    from the Vector engine.
  - tanh runs on the Scalar engine.  Edges are grouped (dependency-exact
    reordering computed on host) so one ACT instruction evaluates up to
    8 edges' tanh from a prescaled staging buffer; the prescale
    (w * state -> staging slot) runs on the Vector engine at 2x mode.
    A greedy balancer decides per-edge between that and a lone
    activation (tanh with free scale) to equalize ACT and DVE load.
  - Two-deep software pipelining: group k's reads depend only on adds
    from groups <= k-2, so every engine streams without stalling.
"""

import sys
import types

import numpy as np

N_NODES = 32
N_INPUTS = 8
N_OUTPUTS = 4
N_EDGES = 128
BATCH = 524288
N_CORES = 8
SHARD = BATCH // N_CORES  # 65536
P = 128
FD = SHARD // P  # 512

N_PSUM = 8          # nodes resident in PSUM (PE-accumulated)
K_BATCH = 8         # max batched-tanh edges per group
K_TOTAL = 10        # max apps per group
LOOKAHEAD = 64      # candidate scan depth when forming a group

# measured per-op engine costs (ns) used by the greedy balancer
C_ACT_LONE = 602.0
C_ACT_BATCH = 440.0
C_DVE_ADD = 600.0
C_DVE_PRESCALE = 333.0
C_DVE_PRESCALE_PSUM = 660.0
C_PE_ADD = 870.0
C_GP_ADD = 1000.0
C_DVE_ADD_PSUM = 658.0


def _install_ntff_hook_shim():
    """The agent image's antenv lacks axon_hooks; recreate it so
    run_bass_kernel_spmd(trace=True) can profile via the axon .so."""
    if "antenv.axon_hooks" in sys.modules:
        return
    mod = types.ModuleType("antenv.axon_hooks")
    mod._hook = None
    mod.set_axon_ntff_profile_hook = lambda h: setattr(mod, "_hook", h)
    mod.get_axon_ntff_profile_hook = lambda: mod._hook
    sys.modules["antenv.axon_hooks"] = mod
    try:
        import antenv

        antenv.axon_hooks = mod
    except ImportError:
        pass
    try:
        from trn_agent_boot.trn_boot import _ntff_profile_via_ctypes

        mod._hook = _ntff_profile_via_ctypes("/opt/axon/libaxon_pjrt.so")
    except Exception:
        pass


def _pruned_apps(src, dst):
    """Exact pruning of the 32x128 sequential edge applications.

    Returns the kept applications in semantic order as (edge_idx, s, d)."""
    nonzero = np.zeros(N_NODES, bool)
    nonzero[:N_INPUTS] = True
    apps = []
    for _ in range(N_NODES):
        for i in range(N_EDGES):
            s, d = int(src[i]), int(dst[i])
            if nonzero[s]:
                apps.append((i, s, d))
                nonzero[d] = True
    live = np.zeros(N_NODES, bool)
    live[N_NODES - N_OUTPUTS:] = True
    keep = []
    for i, s, d in reversed(apps):
        if live[d]:
            keep.append((i, s, d))
            live[s] = True
    keep.reverse()
    return keep


def _choose_psum_nodes(apps):
    in_deg = np.zeros(N_NODES, np.int64)
    for _, _, d in apps:
        in_deg[d] += 1
    return set(np.argsort(-in_deg)[:N_PSUM].tolist())


def _add_engine_map(apps, hot):
    """Static per-node add-engine assignment: hot nodes accumulate on the
    Tensor engine (PSUM); cold nodes split between Vector and GpSimd to
    balance projected load (DVE also carries the prescales)."""
    cnt = np.zeros(N_NODES, np.int64)
    for _, _, d in apps:
        cnt[d] += 1
    eng = {}
    t_pe = 0.0
    for n in hot:
        eng[n] = "pe"
        t_pe += cnt[n] * C_PE_ADD
    cold = [n for n in range(N_NODES) if n not in hot and cnt[n] > 0]
    cold.sort(key=lambda n: -cnt[n])
    t_dve = 0.85e6  # projected prescale load carried by DVE
    t_gp = 0.0
    for n in cold:
        if t_dve + cnt[n] * C_DVE_ADD <= t_gp + cnt[n] * C_GP_ADD:
            eng[n] = "dve"
            t_dve += cnt[n] * C_DVE_ADD
        else:
            eng[n] = "gp"
            t_gp += cnt[n] * C_GP_ADD
    return eng


def _schedule(apps, hot):
    """Group the app list for pipelined emission.

    Returns groups: each is a list of dicts
      {i: semantic index, e: edge idx, s, d, mode: 'lone'|'batch'}.
    Correctness invariants (vs the sequential reference, WAW of adds
    preserved, reads see exactly the semantically-prior adds):
      - app in group k reads its src; all semantically-prior writers of
        that src are in groups <= k-2 (reads of group k are emitted
        before adds of group k-1).
      - an app never jumps ahead of an unscheduled semantically-earlier
        app that writes its src, reads its dst, or writes its dst.
    """
    add_eng = _add_engine_map(apps, hot)
    n = len(apps)
    scheduled = [False] * n
    writer_group = [-10] * N_NODES
    groups = []
    first_un = 0
    n_done = 0
    t_act = 0.0
    t_dve = 0.0
    while n_done < n:
        k = len(groups)
        G = []
        dsts_G = set()
        n_batch = 0
        while first_un < n and scheduled[first_un]:
            first_un += 1
        cnt = 0
        i = first_un
        while i < n and len(G) < K_TOTAL and cnt < LOOKAHEAD:
            if scheduled[i]:
                i += 1
                continue
            cnt += 1
            e, s, d = apps[i]
            ok = writer_group[s] <= k - 2 and s not in dsts_G
            if ok:
                for j in range(first_un, i):
                    if not scheduled[j]:
                        je, js, jd = apps[j]
                        if jd == s or js == d or jd == d:
                            ok = False
                            break
            if ok:
                # engine choice for the tanh
                presc = (C_DVE_PRESCALE_PSUM if s in hot
                         else C_DVE_PRESCALE)
                ae = add_eng[d]
                add_cost = C_DVE_ADD if ae == "dve" else 0.0
                if (n_batch < K_BATCH
                        and max(t_act + C_ACT_BATCH,
                                t_dve + presc + add_cost)
                        < max(t_act + C_ACT_LONE, t_dve + add_cost)):
                    mode = "batch"
                    n_batch += 1
                    t_act += C_ACT_BATCH
                    t_dve += presc + add_cost
                else:
                    mode = "lone"
                    t_act += C_ACT_LONE
                    t_dve += add_cost
                G.append({"i": i, "e": e, "s": s, "d": d, "mode": mode,
                          "ae": ae})
                scheduled[i] = True
                dsts_G.add(d)
                n_done += 1
            i += 1
        # a group with a single batched edge is cheaper as a lone act
        bb = [g for g in G if g["mode"] == "batch"]
        if len(bb) == 1:
            bb[0]["mode"] = "lone"
            t_act += C_ACT_LONE - C_ACT_BATCH
            t_dve -= (C_DVE_PRESCALE_PSUM if bb[0]["s"] in hot
                      else C_DVE_PRESCALE)
        for g in G:
            writer_group[g["d"]] = k
        groups.append(G)  # may be empty: pipeline bubble to advance k
    return groups


def _build_bass(apps, w, hot, want_stats=False):
    import concourse.bacc as bacc
    import concourse.mybir as mybir
    from concourse.tile import TileContext

    f32 = mybir.dt.float32
    Tanh = mybir.ActivationFunctionType.Tanh
    ADD = mybir.AluOpType.add

    groups = _schedule(apps, hot)

    # last add per hot node (for matmul stop flag)
    last_add = {}
    for G in groups:
        for g in G:
            if g["d"] in hot:
                last_add[g["d"]] = g["i"]

    nc = bacc.Bacc("TRN2", target_bir_lowering=False)
    x = nc.dram_tensor("x", [N_INPUTS, P, FD], f32, kind="ExternalInput")
    ident_in = nc.dram_tensor("ident", [P, P], f32, kind="ExternalInput")
    y = nc.dram_tensor("y", [N_OUTPUTS, P, FD], f32, kind="ExternalOutput")

    with TileContext(nc) as tc:
        with tc.tile_pool(name="nodes", bufs=1) as npool, \
             tc.tile_pool(name="tmps", bufs=12) as tpool, \
             tc.tile_pool(name="stage", bufs=3) as spool, \
             tc.tile_pool(name="psum", bufs=1, space="PSUM") as ppool, \
             tc.tile_pool(name="outs", bufs=1) as opool:

            ident = npool.tile([P, P], f32, name="ident", tag="ident")
            nc.sync.dma_start(out=ident, in_=ident_in.ap())
            zero = npool.tile([P, FD], f32, name="zero", tag="zero")
            nc.vector.memset(zero, 0.0)

            node = {}
            for nid in range(N_NODES):
                if nid in hot:
                    node[nid] = ppool.tile([P, FD], f32, name=f"node{nid}",
                                           tag=f"node{nid}")
                else:
                    node[nid] = npool.tile([P, FD], f32, name=f"node{nid}",
                                           tag=f"node{nid}")
            for nid in range(N_NODES):
                if nid < N_INPUTS:
                    if nid in hot:
                        xs = npool.tile([P, FD], f32, name=f"xs{nid}",
                                        tag=f"xs{nid}")
                        nc.sync.dma_start(out=xs, in_=x[nid])
                        nc.tensor.matmul(node[nid], ident, xs, start=True,
                                         stop=False, skip_group_check=True)
                    else:
                        nc.sync.dma_start(out=node[nid], in_=x[nid])
                else:
                    if nid in hot:
                        nc.tensor.matmul(node[nid], ident, zero, start=True,
                                         stop=False, skip_group_check=True)
                    else:
                        nc.vector.memset(node[nid], 0.0)

            def emit_reads(G):
                """prescales (DVE) + lone acts (ACT); returns (stage tile,
                per-app t aps) for the adds phase."""
                batched = [g for g in G if g["mode"] == "batch"]
                taps = {}
                st = None
                if batched:
                    nb = len(batched)
                    st = spool.tile([P, K_BATCH * FD], f32,
                                    name="st", tag="st")
                    for kk, g in enumerate(batched):
                        sl = st[:, kk * FD:(kk + 1) * FD]
                        nc.vector.tensor_scalar_mul(
                            sl, node[g["s"]], float(w[g["e"]]))
                        taps[g["i"]] = sl
                for g in G:
                    if g["mode"] == "lone":
                        t = tpool.tile([P, FD], f32, name="t", tag="t")
                        nc.scalar.activation(t, node[g["s"]], Tanh,
                                             scale=float(w[g["e"]]))
                        taps[g["i"]] = t
                return st, len(batched), taps

            def emit_act(st, nb):
                if st is not None:
                    view = st[:, :nb * FD]
                    nc.scalar.activation(view, view, Tanh)

            def emit_adds(G, taps):
                for g in sorted(G, key=lambda g: g["i"]):
                    t = taps[g["i"]]
                    d = g["d"]
                    if g["ae"] == "pe":
                        nc.tensor.matmul(
                            node[d], ident, t, start=False,
                            stop=(last_add.get(d) == g["i"]),
                            skip_group_check=True)
                    elif g["ae"] == "gp":
                        nc.gpsimd.tensor_tensor(out=node[d], in0=node[d],
                                                in1=t, op=ADD)
                    else:
                        nc.vector.tensor_tensor(out=node[d], in0=node[d],
                                                in1=t, op=ADD)

            prev = None
            for G in groups:
                st, nb, taps = emit_reads(G)
                emit_act(st, nb)
                if prev is not None:
                    emit_adds(*prev)
                prev = (G, taps)
            if prev is not None:
                emit_adds(*prev)

            for j in range(N_OUTPUTS):
                o = opool.tile([P, FD], f32, name=f"out{j}", tag=f"out{j}")
                nc.scalar.activation(o, node[N_NODES - N_OUTPUTS + j], Tanh)
                nc.sync.dma_start(out=y[j], in_=o)
    nc.compile()

    if want_stats:
        n_lone = sum(g["mode"] == "lone" for G in groups for g in G)
        n_batch = sum(g["mode"] == "batch" for G in groups for g in G)
        n_pe = sum(g["ae"] == "pe" for G in groups for g in G)
        n_gp = sum(g["ae"] == "gp" for G in groups for g in G)
        sizes = [len(G) for G in groups if G]
        print(f"schedule: {len(groups)} groups ({sum(1 for G in groups if not G)} bubbles), "
              f"lone={n_lone} batch={n_batch} pe_adds={n_pe} gp_adds={n_gp} "
              f"mean_group={np.mean(sizes):.2f}")
    return nc


def kernel(x, w, src, dst):
    _install_ntff_hook_shim()
    from concourse.bass_utils import run_bass_kernel_spmd

    x = np.asarray(x, dtype=np.float32)
    w = np.asarray(w, dtype=np.float32)
    src = np.asarray(src, dtype=np.int32)
    dst = np.asarray(dst, dtype=np.int32)

    apps = _pruned_apps(src, dst)
    hot = _choose_psum_nodes(apps)
    nc = _build_bass(apps, w, hot)

    in_maps = [
        {"x": np.ascontiguousarray(
            x[:, c * SHARD:(c + 1) * SHARD].reshape(N_INPUTS, P, FD)),
         "ident": np.eye(P, dtype=np.float32)}
        for c in range(N_CORES)
    ]
    res = run_bass_kernel_spmd(nc, in_maps, core_ids=list(range(N_CORES)))
    out = np.concatenate(
        [res.results[c]["y"].reshape(N_OUTPUTS, SHARD) for c in range(N_CORES)],
        axis=1,
    )
    return out


# revision 6
# speedup vs baseline: 1.3860x; 1.3860x over previous
"""Trainium2 Bass kernel for nn_EvolvedNet (gnn_message_passing).

Reference semantics: vals = zeros[32, B]; vals[:8] = x; then 32 sweeps
over 128 edges applied sequentially: vals[dst] += tanh(vals[src] * w);
output = tanh(vals[28:32]).

Strategy (per core, batch shard 65536 = [128 partitions x 512 free] f32):
  - Pure data parallel over 8 NeuronCores.
  - Host-side exact pruning of dead edge applications (source identically
    zero / result cannot reach an output).
  - Each node's state lives in SBUF, except the 8 highest in-degree nodes
    which live in PSUM banks and are accumulated by the Tensor engine
    (fp32 identity-matmul accumulate) - this offloads ~45% of the adds
    from the Vector engine.
  - tanh runs on the Scalar engine.  Edges are grouped (dependency-exact
    reordering computed on host) so one ACT instruction evaluates up to
    8 edges' tanh from a prescaled staging buffer; the prescale
    (w * state -> staging slot) runs on the Vector engine at 2x mode.
    A greedy balancer decides per-edge between that and a lone
    activation (tanh with free scale) to equalize ACT and DVE load.
  - Two-deep software pipelining: group k's reads depend only on adds
    from groups <= k-2, so every engine streams without stalling.
"""

import sys
import types

import numpy as np

N_NODES = 32
N_INPUTS = 8
N_OUTPUTS = 4
N_EDGES = 128
BATCH = 524288
N_CORES = 8
SHARD = BATCH // N_CORES  # 65536
P = 128
FD = SHARD // P  # 512

N_PSUM = 8          # nodes resident in PSUM (PE-accumulated)
K_BATCH = 8         # max batched-tanh edges per group
K_TOTAL = 10        # max apps per group
LOOKAHEAD = 64      # candidate scan depth when forming a group

# measured per-op engine costs (ns) used by the greedy balancer
C_ACT_LONE = 640.0
C_ACT_BATCH = 445.0
C_DVE_ADD = 600.0
C_DVE_PRESCALE = 350.0
C_DVE_PRESCALE_PSUM = 660.0
C_PE_ADD = 1100.0
C_GP_ADD = 1500.0
C_DVE_ADD_PSUM = 658.0
GP_NS_BUDGET = 0.0  # gpsimd SBUF-port contention hurts DVE; keep small/off


def _install_ntff_hook_shim():
    """The agent image's antenv lacks axon_hooks; recreate it so
    run_bass_kernel_spmd(trace=True) can profile via the axon .so."""
    if "antenv.axon_hooks" in sys.modules:
        return
    mod = types.ModuleType("antenv.axon_hooks")
    mod._hook = None
    mod.set_axon_ntff_profile_hook = lambda h: setattr(mod, "_hook", h)
    mod.get_axon_ntff_profile_hook = lambda: mod._hook
    sys.modules["antenv.axon_hooks"] = mod
    try:
        import antenv

        antenv.axon_hooks = mod
    except ImportError:
        pass
    try:
        from trn_agent_boot.trn_boot import _ntff_profile_via_ctypes

        mod._hook = _ntff_profile_via_ctypes("/opt/axon/libaxon_pjrt.so")
    except Exception:
        pass


def _pruned_apps(src, dst):
    """Exact pruning of the 32x128 sequential edge applications.

    Returns the kept applications in semantic order as (edge_idx, s, d)."""
    nonzero = np.zeros(N_NODES, bool)
    nonzero[:N_INPUTS] = True
    apps = []
    for _ in range(N_NODES):
        for i in range(N_EDGES):
            s, d = int(src[i]), int(dst[i])
            if nonzero[s]:
                apps.append((i, s, d))
                nonzero[d] = True
    live = np.zeros(N_NODES, bool)
    live[N_NODES - N_OUTPUTS:] = True
    keep = []
    for i, s, d in reversed(apps):
        if live[d]:
            keep.append((i, s, d))
            live[s] = True
    keep.reverse()
    return keep


def _choose_psum_nodes(apps):
    in_deg = np.zeros(N_NODES, np.int64)
    for _, _, d in apps:
        in_deg[d] += 1
    return set(np.argsort(-in_deg)[:N_PSUM].tolist())


def _add_engine_map(apps, hot):
    """Static per-node add-engine assignment: hot nodes accumulate on the
    Tensor engine (PSUM); cold nodes split between Vector and GpSimd to
    balance projected load (DVE also carries the prescales)."""
    cnt = np.zeros(N_NODES, np.int64)
    for _, _, d in apps:
        cnt[d] += 1
    eng = {}
    for n in hot:
        eng[n] = "pe"
    cold = [n for n in range(N_NODES) if n not in hot and cnt[n] > 0]
    cold.sort(key=lambda n: cnt[n])  # smallest first for the GP budget
    t_gp = 0.0
    for n in cold:
        if t_gp + cnt[n] * C_GP_ADD <= GP_NS_BUDGET:
            eng[n] = "gp"
            t_gp += cnt[n] * C_GP_ADD
        else:
            eng[n] = "dve"
    return eng


def _schedule(apps, hot):
    """Group the app list for pipelined emission.

    Returns groups: each is a list of dicts
      {i: semantic index, e: edge idx, s, d, mode: 'lone'|'batch'}.
    Correctness invariants (vs the sequential reference, WAW of adds
    preserved, reads see exactly the semantically-prior adds):
      - app in group k reads its src; all semantically-prior writers of
        that src are in groups <= k-2 (reads of group k are emitted
        before adds of group k-1).
      - an app never jumps ahead of an unscheduled semantically-earlier
        app that writes its src, reads its dst, or writes its dst.
    """
    add_eng = _add_engine_map(apps, hot)
    n = len(apps)
    scheduled = [False] * n
    writer_group = [-10] * N_NODES
    groups = []
    first_un = 0
    n_done = 0
    t_act = 0.0
    t_dve = 0.0
    t_pe = 0.0
    while n_done < n:
        k = len(groups)
        G = []
        dsts_G = set()
        n_batch = 0
        while first_un < n and scheduled[first_un]:
            first_un += 1
        cnt = 0
        i = first_un
        while i < n and len(G) < K_TOTAL and cnt < LOOKAHEAD:
            if scheduled[i]:
                i += 1
                continue
            cnt += 1
            e, s, d = apps[i]
            ok = writer_group[s] <= k - 2 and s not in dsts_G
            if ok:
                for j in range(first_un, i):
                    if not scheduled[j]:
                        je, js, jd = apps[j]
                        if jd == s or js == d or jd == d:
                            ok = False
                            break
            if ok:
                # engine choice for the tanh
                presc = (C_DVE_PRESCALE_PSUM if s in hot
                         else C_DVE_PRESCALE)
                ae = add_eng[d]
                if ae == "pe" and (t_pe + C_PE_ADD
                                   > t_dve + C_DVE_ADD_PSUM + C_DVE_ADD):
                    ae = "dve_psum"
                if ae == "pe":
                    t_pe += C_PE_ADD
                    add_cost = 0.0
                elif ae == "dve":
                    add_cost = C_DVE_ADD
                elif ae == "dve_psum":
                    add_cost = C_DVE_ADD_PSUM
                else:
                    add_cost = 0.0
                if (n_batch < K_BATCH
                        and max(t_act + C_ACT_BATCH,
                                t_dve + presc + add_cost)
                        < max(t_act + C_ACT_LONE, t_dve + add_cost)):
                    mode = "batch"
                    n_batch += 1
                    t_act += C_ACT_BATCH
                    t_dve += presc + add_cost
                else:
                    mode = "lone"
                    t_act += C_ACT_LONE
                    t_dve += add_cost
                G.append({"i": i, "e": e, "s": s, "d": d, "mode": mode,
                          "ae": ae})
                scheduled[i] = True
                dsts_G.add(d)
                n_done += 1
            i += 1
        # a group with a single batched edge is cheaper as a lone act
        bb = [g for g in G if g["mode"] == "batch"]
        if len(bb) == 1:
            bb[0]["mode"] = "lone"
            t_act += C_ACT_LONE - C_ACT_BATCH
            t_dve -= (C_DVE_PRESCALE_PSUM if bb[0]["s"] in hot
                      else C_DVE_PRESCALE)
        for g in G:
            writer_group[g["d"]] = k
        groups.append(G)  # may be empty: pipeline bubble to advance k
    return groups


def _build_bass(apps, w, hot, want_stats=False):
    import concourse.bacc as bacc
    import concourse.mybir as mybir
    from concourse.tile import TileContext

    f32 = mybir.dt.float32
    Tanh = mybir.ActivationFunctionType.Tanh
    ADD = mybir.AluOpType.add

    groups = _schedule(apps, hot)

    # last PE add per hot node (for matmul stop flag)
    last_add = {}
    for G in groups:
        for g in G:
            if g["ae"] == "pe":
                last_add[g["d"]] = g["i"]

    nc = bacc.Bacc("TRN2", target_bir_lowering=False)
    x = nc.dram_tensor("x", [N_INPUTS, P, FD], f32, kind="ExternalInput")
    ident_in = nc.dram_tensor("ident", [P, P], f32, kind="ExternalInput")
    y = nc.dram_tensor("y", [N_OUTPUTS, P, FD], f32, kind="ExternalOutput")

    with TileContext(nc) as tc:
        with tc.tile_pool(name="nodes", bufs=1) as npool, \
             tc.tile_pool(name="tmps", bufs=16) as tpool, \
             tc.tile_pool(name="stage", bufs=4) as spool, \
             tc.tile_pool(name="psum", bufs=1, space="PSUM") as ppool, \
             tc.tile_pool(name="outs", bufs=1) as opool:

            ident = npool.tile([P, P], f32, name="ident", tag="ident")
            nc.sync.dma_start(out=ident, in_=ident_in.ap())
            zero = npool.tile([P, FD], f32, name="zero", tag="zero")
            nc.vector.memset(zero, 0.0)

            node = {}
            for nid in range(N_NODES):
                if nid in hot:
                    node[nid] = ppool.tile([P, FD], f32, name=f"node{nid}",
                                           tag=f"node{nid}")
                else:
                    node[nid] = npool.tile([P, FD], f32, name=f"node{nid}",
                                           tag=f"node{nid}")
            for nid in range(N_NODES):
                if nid < N_INPUTS:
                    if nid in hot:
                        xs = npool.tile([P, FD], f32, name=f"xs{nid}",
                                        tag=f"xs{nid}")
                        nc.sync.dma_start(out=xs, in_=x[nid])
                        nc.tensor.matmul(node[nid], ident, xs, start=True,
                                         stop=False, skip_group_check=True)
                    else:
                        nc.sync.dma_start(out=node[nid], in_=x[nid])
                else:
                    if nid in hot:
                        nc.tensor.matmul(node[nid], ident, zero, start=True,
                                         stop=False, skip_group_check=True)
                    else:
                        nc.vector.memset(node[nid], 0.0)

            def emit_reads(G):
                """prescales (DVE) + lone acts (ACT); returns (stage tile,
                per-app t aps) for the adds phase."""
                batched = [g for g in G if g["mode"] == "batch"]
                taps = {}
                st = None
                if batched:
                    nb = len(batched)
                    st = spool.tile([P, K_BATCH * FD], f32,
                                    name="st", tag="st")
                    for kk, g in enumerate(batched):
                        sl = st[:, kk * FD:(kk + 1) * FD]
                        nc.vector.tensor_scalar_mul(
                            sl, node[g["s"]], float(w[g["e"]]))
                        taps[g["i"]] = sl
                for g in G:
                    if g["mode"] == "lone":
                        t = tpool.tile([P, FD], f32, name="t", tag="t")
                        nc.scalar.activation(t, node[g["s"]], Tanh,
                                             scale=float(w[g["e"]]))
                        taps[g["i"]] = t
                return st, len(batched), taps

            def emit_act(st, nb):
                if st is not None:
                    view = st[:, :nb * FD]
                    nc.scalar.activation(view, view, Tanh)

            def emit_adds(G, taps):
                for g in sorted(G, key=lambda g: g["i"]):
                    t = taps[g["i"]]
                    d = g["d"]
                    if g["ae"] == "pe":
                        nc.tensor.matmul(
                            node[d], ident, t, start=False,
                            stop=(last_add.get(d) == g["i"]),
                            skip_group_check=True)
                    elif g["ae"] == "gp":
                        nc.gpsimd.tensor_tensor(out=node[d], in0=node[d],
                                                in1=t, op=ADD)
                    else:  # dve / dve_psum
                        nc.vector.tensor_tensor(out=node[d], in0=node[d],
                                                in1=t, op=ADD)

            prev = None
            for G in groups:
                st, nb, taps = emit_reads(G)
                emit_act(st, nb)
                if prev is not None:
                    emit_adds(*prev)
                prev = (G, taps)
            if prev is not None:
                emit_adds(*prev)

            for j in range(N_OUTPUTS):
                o = opool.tile([P, FD], f32, name=f"out{j}", tag=f"out{j}")
                nc.scalar.activation(o, node[N_NODES - N_OUTPUTS + j], Tanh)
                nc.sync.dma_start(out=y[j], in_=o)
    nc.compile()

    if want_stats:
        n_lone = sum(g["mode"] == "lone" for G in groups for g in G)
        n_batch = sum(g["mode"] == "batch" for G in groups for g in G)
        n_pe = sum(g["ae"] == "pe" for G in groups for g in G)
        n_gp = sum(g["ae"] == "gp" for G in groups for g in G)
        sizes = [len(G) for G in groups if G]
        print(f"schedule: {len(groups)} groups ({sum(1 for G in groups if not G)} bubbles), "
              f"lone={n_lone} batch={n_batch} pe_adds={n_pe} gp_adds={n_gp} "
              f"mean_group={np.mean(sizes):.2f}")
    return nc


def kernel(x, w, src, dst):
    _install_ntff_hook_shim()
    from concourse.bass_utils import run_bass_kernel_spmd

    x = np.asarray(x, dtype=np.float32)
    w = np.asarray(w, dtype=np.float32)
    src = np.asarray(src, dtype=np.int32)
    dst = np.asarray(dst, dtype=np.int32)

    apps = _pruned_apps(src, dst)
    hot = _choose_psum_nodes(apps)
    nc = _build_bass(apps, w, hot)

    in_maps = [
        {"x": np.ascontiguousarray(
            x[:, c * SHARD:(c + 1) * SHARD].reshape(N_INPUTS, P, FD)),
         "ident": np.eye(P, dtype=np.float32)}
        for c in range(N_CORES)
    ]
    res = run_bass_kernel_spmd(nc, in_maps, core_ids=list(range(N_CORES)))
    out = np.concatenate(
        [res.results[c]["y"].reshape(N_OUTPUTS, SHARD) for c in range(N_CORES)],
        axis=1,
    )
    return out
